# revision 9
# baseline (speedup 1.0000x reference)
"""Causal self-attention (dense transformer block) on 8 TRN2 NeuronCores.

Tensor-parallel over heads: 16 heads / 8 cores -> 2 heads per core, both
batch elements on every core. All matmuls in bf16 (same PE rate as f32r
at free>=256, about half the power -> less DVFS duty-cycle throttling).

Per core:
  - QKV projection in "T layout": q^T/k^T per head [dh, tok] (bias fused
    into the PSUM eviction), V natural [tok, dh] for both heads
  - causal attention with scores in transposed layout [k, q]: softmax
    numerator via ACT exp (scale folded, sub-range matmuls skip the
    upper-triangle waste); row sums accumulated on DVE; denominator =
    partition-reduce on GPSIMD -> reciprocal row -> rank-1 ones-matmul
    broadcast [dh, tok]; attention output u evicted from PSUM with a
    fused DVE multiply (pre-normalized)
  - out_proj: both heads accumulate into one PSUM bank; eviction is a
    pure copy (alternating ACT/DVE); output bias is added on the host
  - core returns a bf16 partial [2, 2048, 2048]; host sums the 8
    partials and adds the folded bias (v-bias term + b_out)
"""
import sys

if "/opt/trn_rl_repo" not in sys.path:
    sys.path.insert(0, "/opt/trn_rl_repo")

import numpy as np

import concourse.bacc as bacc
import concourse.bass as bass
import concourse.mybir as mybir
import concourse.tile as tile
from concourse.bass_utils import run_bass_kernel_spmd

P = 128
B, S, D = 2, 2048, 2048
H, DH = 16, 128
HPC = 2            # heads per core
NCORES = 8
TC = 256           # token chunk for the QKV projection
QC = 512           # q chunk for attention
SCALE = 1.0 / float(np.sqrt(DH))

f32 = mybir.dt.float32
bf16 = mybir.dt.bfloat16
Act = mybir.ActivationFunctionType
Alu = mybir.AluOpType
AxL = mybir.AxisListType


def _emit(nc, tc_ctx, aps):
    xty, wqkv, bqk, wout, trilm, out_p = aps
    tc = tc_ctx
    NTB = S // P            # 16 token blocks per batch
    NDC = D // P            # 16 contraction chunks
    NQC = S // QC           # 4 q chunks of 512
    NKB = S // P            # 16 key blocks

    with (
        tc.tile_pool(name="const", bufs=1) as const,
        tc.tile_pool(name="xtp", bufs=2) as xtp,
        tc.tile_pool(name="qk", bufs=1) as qk,
        tc.tile_pool(name="vp", bufs=1) as vp,
        tc.tile_pool(name="pp", bufs=4) as pp,
        tc.tile_pool(name="rs", bufs=1) as rs,
        tc.tile_pool(name="dn", bufs=2) as dn,
        tc.tile_pool(name="bcp", bufs=2) as bcp,
        tc.tile_pool(name="up", bufs=1) as up,
        tc.tile_pool(name="fin", bufs=4) as fin,
        tc.tile_pool(name="ps_w", bufs=4, space="PSUM") as ps_w,
        tc.tile_pool(name="ps_u", bufs=4, space="PSUM") as ps_u,
    ):
        bqk_sb = const.tile([P, 4], f32)
        nc.sync.dma_start(bqk_sb, bqk)
        w_sb = const.tile([P, NDC, 6 * P], bf16)
        nc.sync.dma_start(
            w_sb, wqkv.rearrange("(dc p) c -> p dc c", p=P)
        )
        tril_sb = const.tile([P, P], f32)
        ones1 = const.tile([1, P], bf16)
        nc.vector.memset(ones1, 1.0)
        wo_sb = const.tile([P, HPC, D], bf16)

        def load_late_consts():
            nc.sync.dma_start(tril_sb, trilm)
            nc.sync.dma_start(
                wo_sb, wout.rearrange("(h p) c -> p h c", p=P)
            )

        for b in range(B):
            # ---------------- QKV projection ----------------
            q_sb = [qk.tile([P, S], bf16, tag=f"q{h}", name=f"q{h}") for h in range(HPC)]
            k_sb = [qk.tile([P, S], bf16, tag=f"k{h}", name=f"k{h}") for h in range(HPC)]
            v_sb = vp.tile([P, NTB, HPC * DH], bf16, tag="v", name="v_sb")

            for tci in range(S // TC):
                xt = xtp.tile([P, NDC, TC], bf16, tag="xt", name="xt")
                nc.sync.dma_start(
                    xt,
                    xty[b, :, tci * TC:(tci + 1) * TC]
                    .rearrange("(dc p) t -> p dc t", p=P),
                )
                # q^T / k^T for both heads: psum [col=128, tok=TC]
                for cb in range(4):
                    psq = ps_w.tile([P, TC], f32, tag="w", name="psq")
                    for dc in range(NDC):
                        nc.tensor.matmul(
                            psq,
                            w_sb[:, dc, cb * P:(cb + 1) * P],
                            xt[:, dc, :],
                            start=(dc == 0),
                            stop=(dc == NDC - 1),
                        )
                    dst = q_sb[cb] if cb < HPC else k_sb[cb - HPC]
                    nc.scalar.activation(
                        dst[:, tci * TC:(tci + 1) * TC],
                        psq,
                        Act.Identity,
                        bias=bqk_sb[:, cb:cb + 1],
                    )
                # V natural for both heads: psum [tok=128, 2*dh]
                for tb in range(TC // P):
                    psv = ps_w.tile([P, HPC * DH], f32, tag="w", name="psv")
                    for dc in range(NDC):
                        nc.tensor.matmul(
                            psv,
                            xt[:, dc, tb * P:(tb + 1) * P],
                            w_sb[:, dc, 4 * P:6 * P],
                            start=(dc == 0),
                            stop=(dc == NDC - 1),
                        )
                    nc.scalar.copy(v_sb[:, tci * (TC // P) + tb, :], psv)

            if b == 0:
                load_late_consts()

            # ---------------- attention per head ----------------
            u_sb = []
            for h in range(HPC):
                rsum = rs.tile([P, S], f32, tag="rsum", name="rsum")
                us = up.tile([P, S], bf16, tag=f"u{h}", name=f"u{h}")
                psu = [
                    ps_u.tile([P, QC], f32, tag="u", name=f"psu{c}")
                    for c in range(NQC)
                ]
                for kb in range(NKB):
                    c0 = kb // (QC // P)
                    doff = kb * P - c0 * QC
                    for c in range(c0, NQC):
                        psp = ps_w.tile([P, QC], f32, tag="w", name="psp")
                        off = doff if c == c0 else 0
                        nc.tensor.matmul(
                            psp[:, off:],
                            k_sb[h][:, kb * P:(kb + 1) * P],
                            q_sb[h][:, c * QC + off:(c + 1) * QC],
                            start=True,
                            stop=True,
                        )
                        if c == c0:
                            nc.vector.tensor_add(
                                psp[:, doff:doff + P],
                                psp[:, doff:doff + P],
                                tril_sb,
                            )
                        p_t = pp.tile([P, QC], bf16, tag="p", name="p_t")
                        nc.scalar.activation(
                            p_t[:, off:], psp[:, off:], Act.Exp, scale=SCALE
                        )
                        if off > 0:
                            nc.vector.memset(p_t[:, :off], 0.0)
                        if kb == 0:
                            nc.vector.tensor_copy(
                                out=rsum[:, c * QC:(c + 1) * QC], in_=p_t
                            )
                        else:
                            nc.vector.tensor_add(
                                rsum[:, c * QC:(c + 1) * QC],
                                rsum[:, c * QC:(c + 1) * QC],
                                p_t,
                            )
                        nc.tensor.matmul(
                            psu[c],
                            v_sb[:, kb, h * DH:(h + 1) * DH],
                            p_t,
                            start=(kb == 0),
                            stop=(kb == (QC // P) * c + (QC // P) - 1),
                        )
                # denominators: partition-reduce on gpsimd, reciprocal row,
                # rank-1 broadcast to [dh, tok]
                dsum = dn.tile([1, S], f32, tag="dsum", name="dsum")
                nc.gpsimd.tensor_reduce(dsum, rsum, AxL.C, Alu.add)
                rrow = dn.tile([1, S], bf16, tag="rrow", name="rrow")
                nc.vector.reciprocal(rrow, dsum)
                bc = bcp.tile([P, S], bf16, tag="bc", name="bc")
                for c in range(NQC):
                    psb = ps_w.tile([P, QC], f32, tag="w", name="psb")
                    nc.tensor.matmul(
                        psb,
                        ones1,
                        rrow[:, c * QC:(c + 1) * QC],
                        start=True,
                        stop=True,
                    )
                    nc.vector.tensor_copy(out=bc[:, c * QC:(c + 1) * QC], in_=psb)
                # evict + normalize u
                for c in range(NQC):
                    nc.vector.tensor_mul(
                        us[:, c * QC:(c + 1) * QC],
                        psu[c],
                        bc[:, c * QC:(c + 1) * QC],
                    )
                u_sb.append(us)

            # ---------------- out projection ----------------
            for tb in range(NTB):
                for cc in range(D // QC):
                    ps0 = ps_w.tile([P, QC], f32, tag="w", name="ps0")
                    nc.tensor.matmul(
                        ps0,
                        u_sb[0][:, tb * P:(tb + 1) * P],
                        wo_sb[:, 0, cc * QC:(cc + 1) * QC],
                        start=True,
                        stop=False,
                    )
                    nc.tensor.matmul(
                        ps0,
                        u_sb[1][:, tb * P:(tb + 1) * P],
                        wo_sb[:, 1, cc * QC:(cc + 1) * QC],
                        start=False,
                        stop=True,
                    )
                    f_t = fin.tile([P, QC], bf16, tag="fin", name="f_t")
                    if (tb * (D // QC) + cc) % 2 == 0:
                        nc.vector.tensor_copy(out=f_t, in_=ps0)
                    else:
                        nc.scalar.copy(f_t, ps0)
                    nc.sync.dma_start(
                        out_p[b, tb * P:(tb + 1) * P, cc * QC:(cc + 1) * QC],
                        f_t,
                    )


_CACHE = {}


def _build():
    if "nc" in _CACHE:
        return _CACHE["nc"]
    nc = bacc.Bacc("TRN2", target_bir_lowering=False, debug=False)
    xty = nc.dram_tensor("xty", [B, D, S], bf16, kind="ExternalInput").ap()
    wqkv = nc.dram_tensor("wqkv", [D, 6 * P], bf16, kind="ExternalInput").ap()
    bqk = nc.dram_tensor("bqk", [P, 4], f32, kind="ExternalInput").ap()
    wout = nc.dram_tensor("wout", [HPC * DH, D], bf16, kind="ExternalInput").ap()
    trilm = nc.dram_tensor("trilm", [P, P], f32, kind="ExternalInput").ap()
    out_p = nc.dram_tensor("out_p", [B, S, D], bf16, kind="ExternalOutput").ap()
    with nc.allow_low_precision("softmax reciprocal row in bf16 is within budget"):
        with tile.TileContext(nc) as tctx:
            _emit(nc, tctx, (xty, wqkv, bqk, wout, trilm, out_p))
    nc.compile()
    _CACHE["nc"] = nc
    return nc


def _in_maps(x, W_qkv, b_qkv, W_out, b_out):
    import ml_dtypes

    bf = ml_dtypes.bfloat16
    trilm = np.where(
        np.arange(P)[None, :] >= np.arange(P)[:, None], 0.0, -1e9
    ).astype(np.float32)
    xty = np.ascontiguousarray(x.transpose(0, 2, 1)).astype(bf)
    maps = []
    for core in range(NCORES):
        h0 = core * HPC
        cols = []
        for off in (0, D, 2 * D):  # q, k, v column groups of W_qkv
            for h in range(h0, h0 + HPC):
                cols.append((off + h * DH, off + (h + 1) * DH))
        wqkv_c = np.concatenate(
            [W_qkv[:, a:b_] for a, b_ in cols], axis=1
        ).astype(bf)
        bqk_c = np.stack(
            [b_qkv[a:b_] for a, b_ in cols[:4]], axis=1
        ).astype(np.float32)  # [128, 4]
        wout_c = W_out[h0 * DH:(h0 + HPC) * DH, :].astype(bf)
        maps.append({
            "xty": xty,
            "wqkv": np.ascontiguousarray(wqkv_c),
            "bqk": np.ascontiguousarray(bqk_c),
            "wout": np.ascontiguousarray(wout_c),
            "trilm": trilm,
        })
    return maps


def kernel(x, W_qkv, b_qkv, W_out, b_out, _trace=False, _trace_kwargs=None):
    x = np.asarray(x, dtype=np.float32)
    W_qkv = np.asarray(W_qkv, dtype=np.float32)
    b_qkv = np.asarray(b_qkv, dtype=np.float32)
    W_out = np.asarray(W_out, dtype=np.float32)
    b_out = np.asarray(b_out, dtype=np.float32)

    nc = _build()
    maps = _in_maps(x, W_qkv, b_qkv, W_out, b_out)
    res = run_bass_kernel_spmd(
        nc, maps, core_ids=list(range(NCORES)), trace=_trace,
        **(_trace_kwargs or {}),
    )
    out = np.asarray(res.results[0]["out_p"]).astype(np.float32)
    for c in range(1, NCORES):
        out = out + np.asarray(res.results[c]["out_p"]).astype(np.float32)
    # bias (b_out plus the v-bias pushed through W_out) is added on host
    b_v = b_qkv[2 * D:]
    out = out + (b_out + b_v @ W_out)[None, None, :]
    if _trace:
        _CACHE["last_results"] = res
    return out.astype(np.float32)


# revision 15
# speedup vs baseline: 3.2003x; 3.2003x over previous
"""Causal self-attention (dense transformer block) on 8 TRN2 NeuronCores.

Tensor-parallel over heads: 16 heads / 8 cores -> 2 heads per core, both
batch elements on every core. All matmuls in bf16 (same PE rate as f32r
at free>=256, about half the power -> less DVFS duty-cycle throttling).

Per core:
  - QKV projection in "T layout": q^T/k^T per head [dh, tok] (bias fused
    into the PSUM eviction), V natural [tok, dh] for both heads
  - causal attention with scores in transposed layout [k, q]: softmax
    numerator via ACT exp (scale folded, sub-range matmuls skip the
    upper-triangle waste); row sums accumulated on DVE; denominator =
    partition-reduce on GPSIMD -> reciprocal row -> rank-1 ones-matmul
    broadcast [dh, tok]; attention output u evicted from PSUM with a
    fused DVE multiply (pre-normalized)
  - out_proj: both heads accumulate into one PSUM bank; eviction is a
    pure copy (alternating ACT/DVE); output bias is added on the host
  - core returns a bf16 partial [2, 2048, 2048]; host sums the 8
    partials and adds the folded bias (v-bias term + b_out)
"""
import sys

if "/opt/trn_rl_repo" not in sys.path:
    sys.path.insert(0, "/opt/trn_rl_repo")

import numpy as np

import concourse.bacc as bacc
import concourse.bass as bass
import concourse.mybir as mybir
import concourse.tile as tile
from concourse.bass_utils import run_bass_kernel_spmd

P = 128
B, S, D = 2, 2048, 2048
H, DH = 16, 128
HPC = 2            # heads per core
NCORES = 8
TC = 256           # token chunk for the QKV projection
QC = 512           # q chunk for attention
SCALE = 1.0 / float(np.sqrt(DH))

f32 = mybir.dt.float32
bf16 = mybir.dt.bfloat16
Act = mybir.ActivationFunctionType
Alu = mybir.AluOpType
AxL = mybir.AxisListType


def _emit(nc, tc_ctx, aps):
    xty, wqkv, bqk, wout, trilm, ones, idm, out_p = aps
    tc = tc_ctx
    NTB = S // P            # 16 token blocks per batch
    NDC = D // P            # 16 contraction chunks
    NQC = S // QC           # 4 q chunks of 512
    NKB = S // P            # 16 key blocks

    with (
        tc.tile_pool(name="const", bufs=1) as const,
        tc.tile_pool(name="xtp", bufs=2) as xtp,
        tc.tile_pool(name="qk", bufs=1) as qk,
        tc.tile_pool(name="vp", bufs=1) as vp,
        tc.tile_pool(name="pp", bufs=4) as pp,
        tc.tile_pool(name="rs", bufs=1) as rs,
        tc.tile_pool(name="dn", bufs=2) as dn,
        tc.tile_pool(name="bcp", bufs=2) as bcp,
        tc.tile_pool(name="up", bufs=1) as up,
        tc.tile_pool(name="fin", bufs=4) as fin,
        tc.tile_pool(name="ps_w", bufs=4, space="PSUM") as ps_w,
        tc.tile_pool(name="ps_u", bufs=4, space="PSUM") as ps_u,
    ):
        bqk_sb = const.tile([P, 4], f32)
        nc.sync.dma_start(bqk_sb, bqk)
        w_sb = const.tile([P, NDC, 6 * P], bf16)
        nc.sync.dma_start(
            w_sb, wqkv.rearrange("(dc p) c -> p dc c", p=P)
        )
        tril_sb = const.tile([P, P], f32)
        ones_sb = const.tile([P, 1], f32)
        idm_sb = const.tile([P, P], bf16)
        wo_sb = const.tile([P, HPC, D], bf16)

        def load_late_consts():
            nc.sync.dma_start(tril_sb, trilm)
            nc.sync.dma_start(ones_sb, ones)
            nc.sync.dma_start(idm_sb, idm)
            nc.sync.dma_start(
                wo_sb, wout.rearrange("(h p) c -> p h c", p=P)
            )

        for b in range(B):
            # ---------------- QKV projection ----------------
            q_sb = [qk.tile([P, S], bf16, tag=f"q{h}", name=f"q{h}") for h in range(HPC)]
            k_sb = [qk.tile([P, S], bf16, tag=f"k{h}", name=f"k{h}") for h in range(HPC)]
            v_sb = vp.tile([P, NTB, HPC * DH], bf16, tag="v", name="v_sb")

            for tci in range(S // TC):
                xt = xtp.tile([P, NDC, TC], bf16, tag="xt", name="xt")
                nc.sync.dma_start(
                    xt,
                    xty[b, :, tci * TC:(tci + 1) * TC]
                    .rearrange("(dc p) t -> p dc t", p=P),
                )
                # q^T / k^T for both heads: psum [col=128, tok=TC]
                for cb in range(4):
                    psq = ps_w.tile([P, TC], f32, tag="w", name="psq")
                    for dc in range(NDC):
                        nc.tensor.matmul(
                            psq,
                            w_sb[:, dc, cb * P:(cb + 1) * P],
                            xt[:, dc, :],
                            start=(dc == 0),
                            stop=(dc == NDC - 1),
                        )
                    dst = q_sb[cb] if cb < HPC else k_sb[cb - HPC]
                    nc.scalar.activation(
                        dst[:, tci * TC:(tci + 1) * TC],
                        psq,
                        Act.Identity,
                        bias=bqk_sb[:, cb:cb + 1],
                    )
                # V natural for both heads: psum [tok=128, 2*dh]
                for tb in range(TC // P):
                    psv = ps_w.tile([P, HPC * DH], f32, tag="w", name="psv")
                    for dc in range(NDC):
                        nc.tensor.matmul(
                            psv,
                            xt[:, dc, tb * P:(tb + 1) * P],
                            w_sb[:, dc, 4 * P:6 * P],
                            start=(dc == 0),
                            stop=(dc == NDC - 1),
                        )
                    nc.scalar.copy(v_sb[:, tci * (TC // P) + tb, :], psv)

            if b == 0:
                load_late_consts()

            # ---------------- attention per head ----------------
            u_sb = []
            for h in range(HPC):
                rsum = rs.tile([P, S], f32, tag="rsum", name="rsum")
                us = up.tile([P, S], bf16, tag=f"u{h}", name=f"u{h}")
                psu = [
                    ps_u.tile([P, QC], f32, tag="u", name=f"psu{c}")
                    for c in range(NQC)
                ]
                for kb in range(NKB):
                    c0 = kb // (QC // P)
                    doff = kb * P - c0 * QC
                    for c in range(c0, NQC):
                        psp = ps_w.tile([P, QC], f32, tag="w", name="psp")
                        off = doff if c == c0 else 0
                        nc.tensor.matmul(
                            psp[:, off:],
                            k_sb[h][:, kb * P:(kb + 1) * P],
                            q_sb[h][:, c * QC + off:(c + 1) * QC],
                            start=True,
                            stop=True,
                        )
                        if c == c0:
                            nc.vector.tensor_add(
                                psp[:, doff:doff + P],
                                psp[:, doff:doff + P],
                                tril_sb,
                            )
                        p_t = pp.tile([P, QC], bf16, tag="p", name="p_t")
                        nc.scalar.activation(
                            p_t[:, off:], psp[:, off:], Act.Exp, scale=SCALE
                        )
                        if off > 0:
                            nc.vector.memset(p_t[:, :off], 0.0)
                        if kb == 0:
                            nc.vector.tensor_copy(
                                out=rsum[:, c * QC:(c + 1) * QC], in_=p_t
                            )
                        else:
                            nc.vector.tensor_add(
                                rsum[:, c * QC:(c + 1) * QC],
                                rsum[:, c * QC:(c + 1) * QC],
                                p_t,
                            )
                        nc.tensor.matmul(
                            psu[c],
                            v_sb[:, kb, h * DH:(h + 1) * DH],
                            p_t,
                            start=(kb == 0),
                            stop=(kb == (QC // P) * c + (QC // P) - 1),
                        )
                # denominators: per q-block ones-matmul -> [q, 16] psum,
                # reciprocal to bf16, then broadcast matmuls (stride-0
                # lhsT x identity) to bc[dh, tok] = 1/rowsum[tok]
                psr = ps_w.tile([P, NTB], f32, tag="w", name="psr")
                for qb in range(NTB):
                    nc.tensor.matmul(
                        psr[:, qb:qb + 1],
                        rsum[:, qb * P:(qb + 1) * P],
                        ones_sb,
                        start=True,
                        stop=True,
                    )
                ri = dn.tile([P, NTB], bf16, tag="ri", name="ri")
                nc.vector.reciprocal(ri, psr)
                bc = bcp.tile([P, S], bf16, tag="bc", name="bc")
                for c in range(NQC):
                    psb = ps_w.tile([P, QC], f32, tag="w", name="psb")
                    for j in range(QC // P):
                        qb = c * (QC // P) + j
                        nc.tensor.matmul(
                            psb[:, j * P:(j + 1) * P],
                            ri[:, qb:qb + 1].broadcast_to([P, P]),
                            idm_sb,
                            start=True,
                            stop=True,
                        )
                    nc.vector.tensor_copy(out=bc[:, c * QC:(c + 1) * QC], in_=psb)
                # evict + normalize u
                for c in range(NQC):
                    nc.vector.tensor_mul(
                        us[:, c * QC:(c + 1) * QC],
                        psu[c],
                        bc[:, c * QC:(c + 1) * QC],
                    )
                u_sb.append(us)

            # ---------------- out projection ----------------
            for tb in range(NTB):
                for cc in range(D // QC):
                    ps0 = ps_w.tile([P, QC], f32, tag="w", name="ps0")
                    nc.tensor.matmul(
                        ps0,
                        u_sb[0][:, tb * P:(tb + 1) * P],
                        wo_sb[:, 0, cc * QC:(cc + 1) * QC],
                        start=True,
                        stop=False,
                    )
                    nc.tensor.matmul(
                        ps0,
                        u_sb[1][:, tb * P:(tb + 1) * P],
                        wo_sb[:, 1, cc * QC:(cc + 1) * QC],
                        start=False,
                        stop=True,
                    )
                    f_t = fin.tile([P, QC], bf16, tag="fin", name="f_t")
                    if (tb * (D // QC) + cc) % 2 == 0:
                        nc.vector.tensor_copy(out=f_t, in_=ps0)
                    else:
                        nc.scalar.copy(f_t, ps0)
                    nc.sync.dma_start(
                        out_p[b, tb * P:(tb + 1) * P, cc * QC:(cc + 1) * QC],
                        f_t,
                    )


_CACHE = {}


def _build():
    if "nc" in _CACHE:
        return _CACHE["nc"]
    nc = bacc.Bacc("TRN2", target_bir_lowering=False, debug=False)
    xty = nc.dram_tensor("xty", [B, D, S], bf16, kind="ExternalInput").ap()
    wqkv = nc.dram_tensor("wqkv", [D, 6 * P], bf16, kind="ExternalInput").ap()
    bqk = nc.dram_tensor("bqk", [P, 4], f32, kind="ExternalInput").ap()
    wout = nc.dram_tensor("wout", [HPC * DH, D], bf16, kind="ExternalInput").ap()
    trilm = nc.dram_tensor("trilm", [P, P], f32, kind="ExternalInput").ap()
    ones = nc.dram_tensor("ones", [P, 1], f32, kind="ExternalInput").ap()
    idm = nc.dram_tensor("idm", [P, P], bf16, kind="ExternalInput").ap()
    out_p = nc.dram_tensor("out_p", [B, S, D], bf16, kind="ExternalOutput").ap()
    with nc.allow_low_precision("softmax reciprocal in bf16 is within budget"):
        with tile.TileContext(nc) as tctx:
            _emit(nc, tctx, (xty, wqkv, bqk, wout, trilm, ones, idm, out_p))
    nc.compile()
    _CACHE["nc"] = nc
    return nc


def _in_maps(x, W_qkv, b_qkv, W_out, b_out):
    import ml_dtypes

    bf = ml_dtypes.bfloat16
    trilm = np.where(
        np.arange(P)[None, :] >= np.arange(P)[:, None], 0.0, -1e9
    ).astype(np.float32)
    ones = np.ones((P, 1), dtype=np.float32)
    idm = np.eye(P).astype(bf)
    xty = np.ascontiguousarray(x.transpose(0, 2, 1)).astype(bf)
    maps = []
    for core in range(NCORES):
        h0 = core * HPC
        cols = []
        for off in (0, D, 2 * D):  # q, k, v column groups of W_qkv
            for h in range(h0, h0 + HPC):
                cols.append((off + h * DH, off + (h + 1) * DH))
        wqkv_c = np.concatenate(
            [W_qkv[:, a:b_] for a, b_ in cols], axis=1
        ).astype(bf)
        bqk_c = np.stack(
            [b_qkv[a:b_] for a, b_ in cols[:4]], axis=1
        ).astype(np.float32)  # [128, 4]
        wout_c = W_out[h0 * DH:(h0 + HPC) * DH, :].astype(bf)
        maps.append({
            "xty": xty,
            "wqkv": np.ascontiguousarray(wqkv_c),
            "bqk": np.ascontiguousarray(bqk_c),
            "wout": np.ascontiguousarray(wout_c),
            "trilm": trilm,
            "ones": ones,
            "idm": idm,
        })
    return maps


def kernel(x, W_qkv, b_qkv, W_out, b_out, _trace=False, _trace_kwargs=None):
    x = np.asarray(x, dtype=np.float32)
    W_qkv = np.asarray(W_qkv, dtype=np.float32)
    b_qkv = np.asarray(b_qkv, dtype=np.float32)
    W_out = np.asarray(W_out, dtype=np.float32)
    b_out = np.asarray(b_out, dtype=np.float32)

    nc = _build()
    maps = _in_maps(x, W_qkv, b_qkv, W_out, b_out)
    res = run_bass_kernel_spmd(
        nc, maps, core_ids=list(range(NCORES)), trace=_trace,
        **(_trace_kwargs or {}),
    )
    out = np.asarray(res.results[0]["out_p"]).astype(np.float32)
    for c in range(1, NCORES):
        out = out + np.asarray(res.results[c]["out_p"]).astype(np.float32)
    # bias (b_out plus the v-bias pushed through W_out) is added on host
    b_v = b_qkv[2 * D:]
    out = out + (b_out + b_v @ W_out)[None, None, :]
    if _trace:
        _CACHE["last_results"] = res
    return out.astype(np.float32)


# revision 21
# speedup vs baseline: 4.0602x; 1.2687x over previous
"""Causal self-attention (dense transformer block) on 8 TRN2 NeuronCores.

Tensor-parallel over heads: 16 heads / 8 cores -> 2 heads per core, both
batch elements on every core. All matmuls in bf16 (same PE rate as f32r
at free>=256, about half the power -> less DVFS duty-cycle throttling).

Per core:
  - QKV projection in "T layout": q^T/k^T per head [dh, tok] (bias fused
    into the PSUM eviction), V natural [tok, dh] for both heads
  - causal attention with scores in transposed layout [k, q]: sub-range
    matmuls skip the upper-triangle waste on both the scores and the
    attnV accumulation; softmax numerator via ACT exp (scale folded);
    row sums accumulated on DVE; denominator = per-q-block ones-matmul
    -> reciprocal (bf16) -> broadcast matmuls (stride-0 lhsT x identity)
    -> bc[dh, tok]; u evicted from PSUM with a fused DVE multiply
  - out_proj: both heads accumulate into one PSUM bank; eviction is a
    pure copy (alternating ACT/DVE); output bias is added on the host
  - emission is WOVEN to keep the PE busy and the power profile flat:
    attn(b0) interleaved with QKV(b1); attn(b1) interleaved with
    out_proj(b0); out_proj(b1) drains at the end
  - core returns a bf16 partial [2, 2048, 2048]; host sums the 8
    partials and adds the folded bias (v-bias term + b_out)
"""
import sys

if "/opt/trn_rl_repo" not in sys.path:
    sys.path.insert(0, "/opt/trn_rl_repo")

import numpy as np

import concourse.bacc as bacc
import concourse.bass as bass
import concourse.mybir as mybir
import concourse.tile as tile
from concourse.bass_utils import run_bass_kernel_spmd

P = 128
B, S, D = 2, 2048, 2048
H, DH = 16, 128
HPC = 2            # heads per core
NCORES = 8
TC = 512           # token chunk for the QKV projection
QC = 512           # q chunk for attention
SCALE = 1.0 / float(np.sqrt(DH))

f32 = mybir.dt.float32
bf16 = mybir.dt.bfloat16
Act = mybir.ActivationFunctionType

NTB = S // P            # 16 token blocks per batch
NDC = D // P            # 16 contraction chunks
NQC = S // QC           # 4 q chunks of 512
NKB = S // P            # 16 key blocks


def _emit(nc, tc_ctx, aps):
    xty, wqkv, bqk, wout, trilm, ones, idm, out_p = aps
    tc = tc_ctx

    with (
        tc.tile_pool(name="const", bufs=1) as const,
        tc.tile_pool(name="xtp", bufs=2) as xtp,
        tc.tile_pool(name="qk", bufs=1) as qk,
        tc.tile_pool(name="vp", bufs=1) as vp,
        tc.tile_pool(name="pp", bufs=4) as pp,
        tc.tile_pool(name="rs", bufs=1) as rs,
        tc.tile_pool(name="dn", bufs=2) as dn,
        tc.tile_pool(name="bcp", bufs=2) as bcp,
        tc.tile_pool(name="up", bufs=1) as up,
        tc.tile_pool(name="fin", bufs=4) as fin,
        tc.tile_pool(name="ps_w", bufs=4, space="PSUM") as ps_w,
        tc.tile_pool(name="ps_u", bufs=4, space="PSUM") as ps_u,
    ):
        bqk_sb = const.tile([P, 4], f32)
        nc.sync.dma_start(bqk_sb, bqk)
        w_sb = const.tile([P, NDC, 6 * P], bf16)
        nc.sync.dma_start(
            w_sb, wqkv.rearrange("(dc p) c -> p dc c", p=P)
        )
        tril_sb = const.tile([P, P], f32)
        nc.sync.dma_start(tril_sb, trilm)
        ones_sb = const.tile([P, 1], f32)
        nc.sync.dma_start(ones_sb, ones)
        idm_sb = const.tile([P, P], bf16)
        nc.sync.dma_start(idm_sb, idm)
        wo_sb = const.tile([P, HPC, D], bf16)

        st = {}

        def qkv_begin(b):
            st[b] = {
                "q": [qk.tile([P, S], bf16, tag=f"q{h}b{b}", name=f"q{h}b{b}")
                      for h in range(HPC)],
                "k": [qk.tile([P, S], bf16, tag=f"k{h}b{b}", name=f"k{h}b{b}")
                      for h in range(HPC)],
                "v": vp.tile([P, NTB, HPC * DH], bf16, tag=f"vb{b}",
                             name=f"vb{b}"),
                "us": [up.tile([P, S], bf16, tag=f"u{h}b{b}", name=f"u{h}b{b}")
                       for h in range(HPC)],
            }

        def qkv_groups(b):
            """32 psum-group steps of the QKV projection for batch b."""
            s_ = st[b]
            for tci in range(S // TC):
                xt = xtp.tile([P, NDC, TC], bf16, tag="xt", name="xt")
                nc.sync.dma_start(
                    xt,
                    xty[b, :, tci * TC:(tci + 1) * TC]
                    .rearrange("(dc p) t -> p dc t", p=P),
                )
                for cb in range(4):
                    psq = ps_w.tile([P, TC], f32, tag="w", name="psq")
                    for dc in range(NDC):
                        nc.tensor.matmul(
                            psq,
                            w_sb[:, dc, cb * P:(cb + 1) * P],
                            xt[:, dc, :],
                            start=(dc == 0),
                            stop=(dc == NDC - 1),
                        )
                    dst = s_["q"][cb] if cb < HPC else s_["k"][cb - HPC]
                    nc.scalar.activation(
                        dst[:, tci * TC:(tci + 1) * TC],
                        psq,
                        Act.Identity,
                        bias=bqk_sb[:, cb:cb + 1],
                    )
                    yield
                for tb in range(TC // P):
                    psv = ps_w.tile([P, HPC * DH], f32, tag="w", name="psv")
                    for dc in range(NDC):
                        nc.tensor.matmul(
                            psv,
                            xt[:, dc, tb * P:(tb + 1) * P],
                            w_sb[:, dc, 4 * P:6 * P],
                            start=(dc == 0),
                            stop=(dc == NDC - 1),
                        )
                    nc.scalar.copy(
                        s_["v"][:, tci * (TC // P) + tb, :], psv
                    )
                    yield

        def attn_steps(b, h):
            """16 kb steps + a denominator/eviction tail step."""
            s_ = st[b]
            q_sb, k_sb, v_sb = s_["q"][h], s_["k"][h], s_["v"]
            us = s_["us"][h]
            rsum = rs.tile([P, S], f32, tag="rsum", name="rsum")
            psu = [
                ps_u.tile([P, QC], f32, tag="u", name=f"psu{c}")
                for c in range(NQC)
            ]
            for kb in range(NKB):
                c0 = kb // (QC // P)
                doff = kb * P - c0 * QC
                kb_off = kb * P
                p_row = pp.tile([P, S], bf16, tag="p", name="p_row")
                for c in range(c0, NQC):
                    psp = ps_w.tile([P, QC], f32, tag="w", name="psp")
                    off = doff if c == c0 else 0
                    nc.tensor.matmul(
                        psp[:, off:],
                        k_sb[:, kb * P:(kb + 1) * P],
                        q_sb[:, c * QC + off:(c + 1) * QC],
                        start=True,
                        stop=True,
                    )
                    if c == c0:
                        nc.vector.tensor_add(
                            psp[:, doff:doff + P],
                            psp[:, doff:doff + P],
                            tril_sb,
                        )
                    nc.scalar.activation(
                        p_row[:, c * QC + off:(c + 1) * QC],
                        psp[:, off:],
                        Act.Exp,
                        scale=SCALE,
                    )
                if kb == 0:
                    nc.vector.tensor_copy(out=rsum, in_=p_row)
                else:
                    nc.vector.tensor_add(
                        rsum[:, kb_off:], rsum[:, kb_off:], p_row[:, kb_off:]
                    )
                for c in range(c0, NQC):
                    off = doff if c == c0 else 0
                    nc.tensor.matmul(
                        psu[c][:, off:],
                        v_sb[:, kb, h * DH:(h + 1) * DH],
                        p_row[:, c * QC + off:(c + 1) * QC],
                        start=(kb == 0),
                        stop=(kb == (QC // P) * c + (QC // P) - 1),
                        skip_group_check=True,
                    )
                yield
            # denominators: per q-block ones-matmul -> [q, 16] psum,
            # reciprocal to bf16, broadcast matmuls to bc[dh, tok]
            psr = ps_w.tile([P, NTB], f32, tag="w", name="psr")
            for qb in range(NTB):
                nc.tensor.matmul(
                    psr[:, qb:qb + 1],
                    rsum[:, qb * P:(qb + 1) * P],
                    ones_sb,
                    start=True,
                    stop=True,
                )
            ri = dn.tile([P, NTB], bf16, tag="ri", name="ri")
            nc.vector.reciprocal(ri, psr)
            bc = bcp.tile([P, S], bf16, tag="bc", name="bc")
            for c in range(NQC):
                psb = ps_w.tile([P, QC], f32, tag="w", name="psb")
                for j in range(QC // P):
                    qb = c * (QC // P) + j
                    nc.tensor.matmul(
                        psb[:, j * P:(j + 1) * P],
                        ri[:, qb:qb + 1].broadcast_to([P, P]),
                        idm_sb,
                        start=True,
                        stop=True,
                    )
                nc.vector.tensor_copy(
                    out=bc[:, c * QC:(c + 1) * QC], in_=psb
                )
            for c in range(NQC):
                nc.vector.tensor_mul(
                    us[:, c * QC:(c + 1) * QC],
                    psu[c],
                    bc[:, c * QC:(c + 1) * QC],
                )
            yield

        def outproj_tiles(b):
            """64 (tb, cc) output tiles for batch b."""
            s_ = st[b]
            for tb in range(NTB):
                f_t = fin.tile([P, D], bf16, tag="fin", name="f_t")
                for cc in range(D // QC):
                    ps0 = ps_w.tile([P, QC], f32, tag="w", name="ps0")
                    nc.tensor.matmul(
                        ps0,
                        s_["us"][0][:, tb * P:(tb + 1) * P],
                        wo_sb[:, 0, cc * QC:(cc + 1) * QC],
                        start=True,
                        stop=False,
                    )
                    nc.tensor.matmul(
                        ps0,
                        s_["us"][1][:, tb * P:(tb + 1) * P],
                        wo_sb[:, 1, cc * QC:(cc + 1) * QC],
                        start=False,
                        stop=True,
                    )
                    dst = f_t[:, cc * QC:(cc + 1) * QC]
                    if cc % 2 == 0:
                        nc.vector.tensor_copy(out=dst, in_=ps0)
                    else:
                        nc.scalar.copy(dst, ps0)
                    yield
                nc.sync.dma_start(out_p[b, tb * P:(tb + 1) * P, :], f_t)

        def weave(primary, filler, ratio):
            """Emit primary steps; after each, emit `ratio` filler steps
            (fractional, accumulated). Drain both at the end."""
            acc = 0.0
            fit = iter(filler)
            for _ in primary:
                acc += ratio
                while acc >= 1.0:
                    if next(fit, None) is None:
                        acc = 0.0
                        break
                    acc -= 1.0
            for _ in fit:
                pass

        def chain(*gens):
            for g in gens:
                yield from g

        # ---- schedule ----
        qkv_begin(0)
        for _ in qkv_groups(0):
            pass
        nc.sync.dma_start(
            wo_sb, wout.rearrange("(h p) c -> p h c", p=P)
        )
        qkv_begin(1)
        weave(chain(attn_steps(0, 0), attn_steps(0, 1)), qkv_groups(1),
              32 / 34)
        weave(chain(attn_steps(1, 0), attn_steps(1, 1)), outproj_tiles(0),
              64 / 34)
        for _ in outproj_tiles(1):
            pass


_CACHE = {}


def _build():
    if "nc" in _CACHE:
        return _CACHE["nc"]
    nc = bacc.Bacc("TRN2", target_bir_lowering=False, debug=False)
    xty = nc.dram_tensor("xty", [B, D, S], bf16, kind="ExternalInput").ap()
    wqkv = nc.dram_tensor("wqkv", [D, 6 * P], bf16, kind="ExternalInput").ap()
    bqk = nc.dram_tensor("bqk", [P, 4], f32, kind="ExternalInput").ap()
    wout = nc.dram_tensor("wout", [HPC * DH, D], bf16, kind="ExternalInput").ap()
    trilm = nc.dram_tensor("trilm", [P, P], f32, kind="ExternalInput").ap()
    ones = nc.dram_tensor("ones", [P, 1], f32, kind="ExternalInput").ap()
    idm = nc.dram_tensor("idm", [P, P], bf16, kind="ExternalInput").ap()
    out_p = nc.dram_tensor("out_p", [B, S, D], bf16, kind="ExternalOutput").ap()
    with nc.allow_low_precision("softmax reciprocal in bf16 is within budget"):
        with tile.TileContext(nc) as tctx:
            _emit(nc, tctx, (xty, wqkv, bqk, wout, trilm, ones, idm, out_p))
    nc.compile()
    _CACHE["nc"] = nc
    return nc


def _in_maps(x, W_qkv, b_qkv, W_out, b_out):
    import ml_dtypes

    bf = ml_dtypes.bfloat16
    trilm = np.where(
        np.arange(P)[None, :] >= np.arange(P)[:, None], 0.0, -1e9
    ).astype(np.float32)
    ones = np.ones((P, 1), dtype=np.float32)
    idm = np.eye(P).astype(bf)
    xty = np.ascontiguousarray(x.transpose(0, 2, 1)).astype(bf)
    maps = []
    for core in range(NCORES):
        h0 = core * HPC
        cols = []
        for off in (0, D, 2 * D):  # q, k, v column groups of W_qkv
            for h in range(h0, h0 + HPC):
                cols.append((off + h * DH, off + (h + 1) * DH))
        wqkv_c = np.concatenate(
            [W_qkv[:, a:b_] for a, b_ in cols], axis=1
        ).astype(bf)
        bqk_c = np.stack(
            [b_qkv[a:b_] for a, b_ in cols[:4]], axis=1
        ).astype(np.float32)  # [128, 4]
        wout_c = W_out[h0 * DH:(h0 + HPC) * DH, :].astype(bf)
        maps.append({
            "xty": xty,
            "wqkv": np.ascontiguousarray(wqkv_c),
            "bqk": np.ascontiguousarray(bqk_c),
            "wout": np.ascontiguousarray(wout_c),
            "trilm": trilm,
            "ones": ones,
            "idm": idm,
        })
    return maps


def kernel(x, W_qkv, b_qkv, W_out, b_out, _trace=False, _trace_kwargs=None):
    x = np.asarray(x, dtype=np.float32)
    W_qkv = np.asarray(W_qkv, dtype=np.float32)
    b_qkv = np.asarray(b_qkv, dtype=np.float32)
    W_out = np.asarray(W_out, dtype=np.float32)
    b_out = np.asarray(b_out, dtype=np.float32)

    nc = _build()
    maps = _in_maps(x, W_qkv, b_qkv, W_out, b_out)
    res = run_bass_kernel_spmd(
        nc, maps, core_ids=list(range(NCORES)), trace=_trace,
        **(_trace_kwargs or {}),
    )
    out = np.asarray(res.results[0]["out_p"]).astype(np.float32)
    for c in range(1, NCORES):
        out = out + np.asarray(res.results[c]["out_p"]).astype(np.float32)
    # bias (b_out plus the v-bias pushed through W_out) is added on host
    b_v = b_qkv[2 * D:]
    out = out + (b_out + b_v @ W_out)[None, None, :]
    if _trace:
        _CACHE["last_results"] = res
    return out.astype(np.float32)
